# revision 38
# baseline (speedup 1.0000x reference)
"""Trainium2 Bass kernel for a batched binary-tree (child-sum-ish) LSTM cell.

Computes, for N=8192 nodes (D = HD = 1024):
    z   = sigmoid([x_l x_r] @ W_z.T + b_z)
    x_t = z * x_l + (1-z) * x_r
    [x_i x_f x_o x_g] = x_t @ W_xin.T
    i   = sigmoid([h_l h_r c_l c_r] @ W_i.T  + b_i  + x_i)
    f_l = sigmoid([h_l h_r c_l c_r] @ W_fl.T + b_fl + x_f)
    f_r = sigmoid([h_l h_r c_l c_r] @ W_fr.T + b_fr + x_f)
    g   = tanh   ([h_l h_r]         @ W_g.T  + b_g  + x_g)
    c_t = f_l*c_l + f_r*c_r + i*g
    o   = sigmoid([h_l h_r c_t]     @ W_o.T  + b_o  + x_o)
    h_t = o * tanh(c_t)
returns (x_t, h_t, c_t).

Strategy: data-parallel over 8 NeuronCores (1024 rows each). All work is done
in transposed space (features on SBUF partitions, rows on the free dim) so no
on-device transposes are needed; the host pre-transposes activations and
pre-tiles weights. Matmuls run in bf16 with fp32 PSUM accumulate; elementwise
math runs in fp32. The kernel is PE-roofline-bound (~215 ns per 128x128x512
bf16 matmul, 2944 matmuls/core); the schedule below exists to keep the PE fed
from the first microsecond to the last:
  - a short burst of dummy matmuls on a memset tile warms the PE HAM clock
    gate during the initial DMA window, so real matmuls start at 2.4 GHz;
  - the first z-weight tile and the slab-0 x loads are split into small
    chunks so the first real matmul issues as soon as ~1/4 of the data is in;
  - the shared x_f projection for all 8 m-tiles runs as a prelude between
    phases A and B (it needs only x_t), giving the h/c resident loads an
    extra ~14 us of PE work to hide behind;
  - c_l/c_r for the fp32 elementwise path are shipped as bf16 hi (shared
    with the matmul operand) + bf16 residual and reconstructed on the DVE,
    which halves their DMA footprint in the startup crunch;
  - the next slab's x prefetch is deferred behind this slab's critical loads;
  - the final o-gate m-tile is post-processed in two row-halves with the
    stores split across the sync+scalar rings to shorten the serial tail.
Weight streaming uses the sync HWDGE ring, activation loads the SWDGE ring,
and output stores the scalar HWDGE ring, so the three flows never FIFO-block
each other (stores never ride the weight ring except after its last load).
"""

import sys

if "/opt/trn_rl_repo" not in sys.path:
    sys.path.insert(0, "/opt/trn_rl_repo")

import numpy as np
import ml_dtypes

N_CORES = 8
N = 8192
D = 1024
P = 128
NL = N // N_CORES          # rows per core
KB = D // P                # 8 k-blocks per 1024-feature tensor
MT = D // P                # 8 output m-tiles per gate

# (name, K-tiles, bias index, act fn, [(rhs part, weight k-tile offset)],
#  add_xf). The x_i/x_g projections ride the i/g PSUM accumulations; the
# shared x_f is computed for all m-tiles in a prelude and DVE-added into
# both f-gate PSUMs.
_GATES_B = [
    ("i",  40, 1, "sig",  [("xtb", 32), ("hbl", 0), ("hbr", 8),
                           ("cbl", 16), ("cbr", 24)], False),
    ("fl", 32, 2, "sig",  [("hbl", 0), ("hbr", 8), ("cbl", 16), ("cbr", 24)], True),
    ("fr", 32, 3, "sig",  [("hbl", 0), ("hbr", 8), ("cbl", 16), ("cbr", 24)], True),
    ("g",  24, 4, "tanh", [("xtb", 16), ("hbl", 0), ("hbr", 8)], False),
]

N_WARM = 12                # dummy matmuls to hold the PE clock gate open

_compiled = {}


def _build(R):
    """Build + compile the per-core Bass program. R = rows per slab."""
    import concourse.mybir as mybir
    import concourse.tile as tile
    from concourse import bacc

    F32 = mybir.dt.float32
    F16 = mybir.dt.float16
    BF16 = mybir.dt.bfloat16
    SIG = mybir.ActivationFunctionType.Sigmoid
    TANH = mybir.ActivationFunctionType.Tanh

    assert NL % R == 0
    n_slabs = NL // R
    H = R // 2

    nc = bacc.Bacc("TRN2", target_bir_lowering=False, debug=False)

    def din(name, shape, dt):
        return nc.dram_tensor(name, shape, dt, kind="ExternalInput").ap()

    def dout(name, shape, dt):
        return nc.dram_tensor(name, shape, dt, kind="ExternalOutput").ap()

    # Transposed activations [D, NL], bf16 for the matmul path. c residuals
    # (clo = bf16(c - bf16(c))) rebuild fp32-grade c for the elementwise path.
    xb_l = din("xb_l", [D, NL], BF16)
    xb_r = din("xb_r", [D, NL], BF16)
    hb_l = din("hb_l", [D, NL], BF16)
    hb_r = din("hb_r", [D, NL], BF16)
    cb_l = din("cb_l", [D, NL], BF16)
    cb_r = din("cb_r", [D, NL], BF16)
    clo_l = din("clo_l", [D, NL], BF16)
    clo_r = din("clo_r", [D, NL], BF16)
    # Weights pre-tiled on host to [MT, P, Kt, P] (partition-major so each
    # per-partition DMA run is Kt*256B contiguous).
    wz = din("wz", [MT, P, 16, P], BF16)
    wi = din("wi", [MT, P, 40, P], BF16)
    wfl = din("wfl", [MT, P, 32, P], BF16)
    wfr = din("wfr", [MT, P, 32, P], BF16)
    wg = din("wg", [MT, P, 24, P], BF16)
    wo = din("wo", [MT, P, 32, P], BF16)
    wxf = din("wxf", [MT, P, 8, P], BF16)
    wmap = {"i": wi, "fl": wfl, "fr": wfr, "g": wg}
    bias = din("bias", [P, 6, MT], F32)

    xT_o = dout("xT_o", [D, NL], F32)
    hT_o = dout("hT_o", [D, NL], F32)
    cT_o = dout("cT_o", [D, NL], F32)

    def r3(ap):
        return ap.rearrange("(k p) n -> p k n", p=P)

    with tile.TileContext(nc) as tc:
        with (
            tc.tile_pool(name="acts", bufs=1) as apool,
            tc.tile_pool(name="w", bufs=4) as wpool,
            tc.tile_pool(name="gates", bufs=8) as gpool,
            tc.tile_pool(name="work", bufs=7) as wkpool,
            tc.tile_pool(name="ps", bufs=8, space="PSUM") as pspool,
            tc.tile_pool(name="cst", bufs=1) as cpool,
        ):
            bias_t = cpool.tile([P, 6, MT], F32, name="bias_t")
            nc.sync.dma_start(bias_t[:], bias[:])

            # PE warmup: the HAM clock gate needs ~3.4us of sustained PE
            # activity to lift the PE from 1.2 to 2.4 GHz, and re-throttles
            # after ~3.4us idle. These dummies span the initial DMA window.
            warm_t = cpool.tile([P, 512], BF16, name="warm_t")
            nc.vector.memset(warm_t[:], 0.0)
            ps_warm = pspool.tile([P, R], F32, tag="ps", name="ps_warm")
            for _ in range(N_WARM):
                nc.tensor.matmul(ps_warm[:, :512], warm_t[:, :P], warm_t[:],
                                 start=True, stop=True)

            def load_xb(s):
                rs_ = slice(s * R, s * R + R)
                xl = apool.tile([P, KB, R], BF16, tag="xbl", name="xbl")
                xr = apool.tile([P, KB, R], BF16, tag="xbr", name="xbr")
                if s == 0:
                    # Fine chunks so the very first matmuls' k-blocks land
                    # as early as possible.
                    for j in range(0, KB, 2):
                        nc.gpsimd.dma_start(xl[:, j:j + 2, :],
                                            r3(xb_l)[:, j:j + 2, rs_])
                    for j in range(0, KB, 2):
                        nc.scalar.dma_start(xr[:, j:j + 2, :],
                                            r3(xb_r)[:, j:j + 2, rs_])
                else:
                    nc.gpsimd.dma_start(xl[:, :2, :], r3(xb_l)[:, :2, rs_])
                    nc.gpsimd.dma_start(xl[:, 2:, :], r3(xb_l)[:, 2:, rs_])
                    nc.scalar.dma_start(xr[:], r3(xb_r)[:, :, rs_])
                return xl, xr

            next_xb = load_xb(0)
            for s in range(n_slabs):
                r0 = s * R
                rs = slice(r0, r0 + R)

                xbl_t, xbr_t = next_xb
                # Residents for phases B/C. Tiles are allocated here but the
                # loads are issued one per phase-A m-tile from the scalar
                # engine stream: each sits behind that m-tile's activation
                # semaphore wait, which paces the queue to ~1 MB per m-tile.
                # An unthrottled burst would drop the sync queue's SDMA
                # round-robin share below the z-weight feed rate and starve
                # the PE (the DMA engines share bandwidth equally between
                # queues that have pending work).
                res_tiles = {
                    n: apool.tile([P, KB, R], BF16, tag=n, name=n)
                    for n in ("hbl", "hbr", "cbl", "cbr", "clol", "clor")
                }
                res_dram = {"hbl": hb_l, "hbr": hb_r, "cbl": cb_l,
                            "cbr": cb_r, "clol": clo_l, "clor": clo_r}
                hbl_t, hbr_t = res_tiles["hbl"], res_tiles["hbr"]
                cbl_t, cbr_t = res_tiles["cbl"], res_tiles["cbr"]
                clol_t, clor_t = res_tiles["clol"], res_tiles["clor"]

                xtb_t = apool.tile([P, KB, R], BF16, tag="xtb", name="xtb")
                ctb_t = apool.tile([P, KB, R], BF16, tag="ctb", name="ctb")
                tct_t = apool.tile([P, KB, R], F16, tag="tct", name="tct")

                # z weights m=5..7 ride the scalar HWDGE queue (loaded ahead
                # of phase A) so the sync queue's pre-B critical path is just
                # wz[0..4] + B m=0's weights.
                wzs = []
                for m in range(5, MT):
                    t = wpool.tile([P, 16, P], BF16, tag="wzs", bufs=3,
                                   name="wzs_t")
                    nc.scalar.dma_start(t[:], wz[m])
                    wzs.append(t)
                # cbl/cbr ride the gpsimd queue (un-paceable there, but at
                # 2 MB the steal from the weight queue is tolerable and they
                # arrive well before B m=0's c k-blocks need them).
                nc.gpsimd.dma_start(cbl_t[:], r3(cb_l)[:, :, rs])
                nc.gpsimd.dma_start(cbr_t[:], r3(cb_r)[:, :, rs])

                # ---- Phase A: z gate + x_t ----
                for m in range(MT):
                    if m >= 5:
                        w_t = wzs[m - 5]
                    else:
                        w_t = wpool.tile([P, 16, P], BF16, tag="wz", bufs=3,
                                         name="wz_t")
                        if s == 0 and m == 0:
                            for c in range(0, 16, 4):
                                nc.sync.dma_start(w_t[:, c:c + 4, :],
                                                  wz[0][:, c:c + 4, :])
                        else:
                            nc.sync.dma_start(w_t[:], wz[m])
                    ps = pspool.tile([P, R], F32, tag="ps", name="ps_z")
                    for kt in range(16):
                        rhs = (xbl_t if kt < KB else xbr_t)[:, kt % KB, :]
                        nc.tensor.matmul(ps[:], w_t[:, kt, :], rhs,
                                         start=(kt == 0), stop=(kt == 15))
                    z_t = wkpool.tile([P, R], F32, tag="wk", name="z_t")
                    nc.scalar.activation(z_t[:], ps[:], SIG, bias=bias_t[:, 0, m, None])
                    d_t = wkpool.tile([P, R], F32, tag="wk", name="d_t")
                    nc.vector.tensor_sub(d_t[:], xbl_t[:, m, :], xbr_t[:, m, :])
                    nc.vector.tensor_mul(d_t[:], d_t[:], z_t[:])
                    xt_m = wkpool.tile([P, R], F32, tag="wk", name="xt_m")
                    nc.vector.tensor_add(xt_m[:], d_t[:], xbr_t[:, m, :])
                    nc.scalar.dma_start(r3(xT_o)[:, m, rs], xt_m[:])
                    nc.vector.tensor_copy(xtb_t[:, m, :], xt_m[:])
                    # Paced h/clo loads: one per phase-A m-tile, sitting
                    # behind this m-tile's activation in the scalar stream.
                    if m < 4:
                        rn = ("hbl", "hbr", "clol", "clor")[m]
                        nc.scalar.dma_start(res_tiles[rn][:],
                                            r3(res_dram[rn])[:, :, rs])

                parts = {"hbl": hbl_t, "hbr": hbr_t, "cbl": cbl_t,
                         "cbr": cbr_t, "xtb": xtb_t, "ctb": ctb_t}

                # ---- Phase B: i, f_l, f_r, g gates + c_t. The shared x_f
                # projection rides a per-m matmul into SBUF (its weights on a
                # dedicated 8-slot tag so their loads never WAR-stall the
                # sync engine); the i-gate leads each m-tile with 8 x_t-only
                # k-blocks of runway before touching h/c. ----
                for m in range(MT):
                    wxf_m = wpool.tile([P, KB, P], BF16, tag="wxf", bufs=8,
                                       name="wxf_m")
                    nc.sync.dma_start(wxf_m[:], wxf[m])
                    ps_xf = pspool.tile([P, R], F32, tag="ps", name="ps_xf")
                    for j in range(KB):
                        nc.tensor.matmul(ps_xf[:], wxf_m[:, j, :],
                                         xtb_t[:, j, :],
                                         start=(j == 0), stop=(j == KB - 1))
                    xfp_m = gpool.tile([P, R], F32, tag="gate", name="xfp_m")
                    nc.scalar.copy(xfp_m[:], ps_xf[:])
                    gt = {}
                    for (gname, Kt, b_idx, fn, rparts, add_xf) in _GATES_B:
                        w_t = wpool.tile([P, Kt, P], BF16, tag="w",
                                         name=f"w_{gname}")
                        nc.sync.dma_start(w_t[:], wmap[gname][m])
                        ps = pspool.tile([P, R], F32, tag="ps",
                                         name=f"ps_{gname}")
                        n_done = 0
                        for (pname, koff) in rparts:
                            pt = parts[pname]
                            for j in range(KB):
                                nc.tensor.matmul(
                                    ps[:], w_t[:, koff + j, :], pt[:, j, :],
                                    start=(n_done == 0),
                                    stop=(n_done == Kt - 1))
                                n_done += 1
                        if add_xf:
                            nc.vector.tensor_add(ps[:], ps[:], xfp_m[:])
                        g_t = gpool.tile([P, R], F32, tag="gate",
                                         name=f"g_{gname}")
                        nc.scalar.activation(
                            g_t[:], ps[:], SIG if fn == "sig" else TANH,
                            bias=bias_t[:, b_idx, m, None])
                        gt[gname] = g_t
                    # Rebuild fp32-grade c_l/c_r from bf16 hi + bf16 residual.
                    cfl_m = wkpool.tile([P, R], F32, tag="wk", name="cfl_m")
                    nc.vector.tensor_add(cfl_m[:], cbl_t[:, m, :], clol_t[:, m, :])
                    cfr_m = wkpool.tile([P, R], F32, tag="wk", name="cfr_m")
                    nc.vector.tensor_add(cfr_m[:], cbr_t[:, m, :], clor_t[:, m, :])
                    ct_m = wkpool.tile([P, R], F32, tag="wk", name="ct_m")
                    nc.vector.tensor_mul(ct_m[:], gt["fl"][:], cfl_m[:])
                    t2 = wkpool.tile([P, R], F32, tag="wk", name="t2")
                    nc.vector.tensor_mul(t2[:], gt["fr"][:], cfr_m[:])
                    nc.vector.tensor_add(ct_m[:], ct_m[:], t2[:])
                    nc.vector.tensor_mul(t2[:], gt["i"][:], gt["g"][:])
                    nc.vector.tensor_add(ct_m[:], ct_m[:], t2[:])
                    nc.scalar.dma_start(r3(cT_o)[:, m, rs], ct_m[:])
                    nc.vector.tensor_copy(ctb_t[:, m, :], ct_m[:])
                    nc.scalar.activation(tct_t[:, m, :], ct_m[:], TANH)
                    if m == 1 and s + 1 < n_slabs:
                        # Deferred so it never competes with this slab's
                        # critical startup loads.
                        next_xb = load_xb(s + 1)

                # ---- Phase C: o gate + h_t ----
                for m in range(MT):
                    # wo rides the (otherwise idle) SWDGE ring: its pool-slot
                    # WAR waits then stall the gpsimd engine, not the sync
                    # engine, so the next slab's z-weight dma_starts issue
                    # as soon as phase B's last gate-weight load is in.
                    w_t = wpool.tile([P, 32, P], BF16, tag="w", name="wo_t")
                    nc.gpsimd.dma_start(w_t[:], wo[m])
                    ps = pspool.tile([P, R], F32, tag="ps", name="ps_o")
                    kt = 0
                    for pname in ["hbl", "hbr", "ctb", "xtb"]:
                        pt = parts[pname]
                        for j in range(KB):
                            nc.tensor.matmul(ps[:], w_t[:, kt, :], pt[:, j, :],
                                             start=(kt == 0), stop=(kt == 31))
                            kt += 1
                    last = (s == n_slabs - 1 and m == MT - 1)
                    if not last:
                        o_t = wkpool.tile([P, R], F32, tag="wk", name="o_t")
                        nc.scalar.activation(o_t[:], ps[:], SIG,
                                             bias=bias_t[:, 5, m, None])
                        ht_m = wkpool.tile([P, R], F32, tag="wk", name="ht_m")
                        nc.vector.tensor_mul(ht_m[:], o_t[:], tct_t[:, m, :])
                        nc.scalar.dma_start(r3(hT_o)[:, m, rs], ht_m[:])
                    else:
                        # Tail: two row-halves, stores split across the sync
                        # (now weight-idle) and scalar rings.
                        for hi, eng in ((0, nc.sync), (1, nc.scalar)):
                            hs = slice(hi * H, (hi + 1) * H)
                            rh = slice(r0 + hi * H, r0 + (hi + 1) * H)
                            o_h = wkpool.tile([P, H], F32, tag="wkh", bufs=4,
                                              name="o_h")
                            nc.scalar.activation(o_h[:], ps[:, hs], SIG,
                                                 bias=bias_t[:, 5, m, None])
                            ht_h = wkpool.tile([P, H], F32, tag="wkh", bufs=4,
                                               name="ht_h")
                            nc.vector.tensor_mul(ht_h[:], o_h[:],
                                                 tct_t[:, m, hs])
                            eng.dma_start(r3(hT_o)[:, m, rh], ht_h[:])

    nc.compile()
    return nc


def _get_compiled(R=512):
    if R not in _compiled:
        _compiled[R] = _build(R)
    return _compiled[R]


def _prep_weight(w_km):
    """[K, D] (K-major stack of W.T blocks) -> [MT, P, Kt, P] bf16."""
    K = w_km.shape[0]
    kt = K // P
    w = w_km.reshape(kt, P, MT, P)          # [kt, p, m, f]
    w = np.ascontiguousarray(w.transpose(2, 1, 0, 3))  # [m, p, kt, f]
    return w.astype(ml_dtypes.bfloat16)


def _host_prep(inp):
    """Transpose/stack/cast everything the device program wants."""
    f32 = np.float32
    t = {k: np.ascontiguousarray(np.asarray(inp[k], dtype=f32).T)
         for k in ("x_l", "x_r", "h_l", "h_r", "c_l", "c_r")}
    bf = {k: v.astype(ml_dtypes.bfloat16) for k, v in t.items()}
    # c residuals: c ~= bf16(c) + bf16(c - bf16(c)) to ~2^-17 relative.
    clo = {k: (t[k] - bf[k].astype(f32)).astype(ml_dtypes.bfloat16)
           for k in ("c_l", "c_r")}

    W_i = np.asarray(inp["W_i"], f32)
    W_fl = np.asarray(inp["W_fl"], f32)
    W_fr = np.asarray(inp["W_fr"], f32)
    W_xin = np.asarray(inp["W_xin"], f32)
    W_o = np.asarray(inp["W_o"], f32)
    W_z = np.asarray(inp["W_z"], f32)
    W_g = np.asarray(inp["W_g"], f32)

    x_i = W_xin[0 * D:1 * D].T    # [D, D] blocks of W_xin.T
    x_f = W_xin[1 * D:2 * D].T
    x_o = W_xin[2 * D:3 * D].T
    x_g = W_xin[3 * D:4 * D].T

    weights = {
        "wz": _prep_weight(np.ascontiguousarray(W_z.T)),
        "wi": _prep_weight(np.concatenate([W_i.T, x_i], axis=0)),
        "wfl": _prep_weight(np.ascontiguousarray(W_fl.T)),
        "wfr": _prep_weight(np.ascontiguousarray(W_fr.T)),
        "wg": _prep_weight(np.concatenate([W_g.T, x_g], axis=0)),
        "wo": _prep_weight(np.concatenate([W_o.T, x_o], axis=0)),
        "wxf": _prep_weight(x_f),
    }

    b = np.stack([np.asarray(inp[k], f32) for k in
                  ("b_z", "b_i", "b_fl", "b_fr", "b_g", "b_o")])  # [6, D]
    bias = np.ascontiguousarray(b.reshape(6, MT, P).transpose(2, 0, 1))

    in_maps = []
    for c in range(N_CORES):
        cs = slice(c * NL, (c + 1) * NL)
        m = {
            "xb_l": np.ascontiguousarray(bf["x_l"][:, cs]),
            "xb_r": np.ascontiguousarray(bf["x_r"][:, cs]),
            "hb_l": np.ascontiguousarray(bf["h_l"][:, cs]),
            "hb_r": np.ascontiguousarray(bf["h_r"][:, cs]),
            "cb_l": np.ascontiguousarray(bf["c_l"][:, cs]),
            "cb_r": np.ascontiguousarray(bf["c_r"][:, cs]),
            "clo_l": np.ascontiguousarray(clo["c_l"][:, cs]),
            "clo_r": np.ascontiguousarray(clo["c_r"][:, cs]),
            "bias": bias,
        }
        m.update(weights)
        in_maps.append(m)
    return in_maps


def run(inputs, R=512, trace=False, trace_kwargs=None):
    """Run on 8 cores; returns (results, BassKernelResults)."""
    from concourse.bass_utils import run_bass_kernel_spmd

    if trace:
        try:
            from hookfix import install_ntff_hook
            install_ntff_hook()
        except Exception:
            pass
    nc = _get_compiled(R)
    in_maps = _host_prep(inputs)
    res = run_bass_kernel_spmd(nc, in_maps, core_ids=list(range(N_CORES)),
                               trace=trace, **(trace_kwargs or {}))
    xT = np.concatenate([res.results[c]["xT_o"] for c in range(N_CORES)], axis=1)
    hT = np.concatenate([res.results[c]["hT_o"] for c in range(N_CORES)], axis=1)
    cT = np.concatenate([res.results[c]["cT_o"] for c in range(N_CORES)], axis=1)
    x_t = np.ascontiguousarray(xT.T)
    h_t = np.ascontiguousarray(hT.T)
    c_t = np.ascontiguousarray(cT.T)
    return (x_t, h_t, c_t), res


def kernel(**inputs):
    out, _ = run(inputs)
    return out


# revision 48
# speedup vs baseline: 1.0232x; 1.0232x over previous
"""Trainium2 Bass kernel for a batched binary-tree (child-sum-ish) LSTM cell.

Computes, for N=8192 nodes (D = HD = 1024):
    z   = sigmoid([x_l x_r] @ W_z.T + b_z)
    x_t = z * x_l + (1-z) * x_r
    [x_i x_f x_o x_g] = x_t @ W_xin.T
    i   = sigmoid([h_l h_r c_l c_r] @ W_i.T  + b_i  + x_i)
    f_l = sigmoid([h_l h_r c_l c_r] @ W_fl.T + b_fl + x_f)
    f_r = sigmoid([h_l h_r c_l c_r] @ W_fr.T + b_fr + x_f)
    g   = tanh   ([h_l h_r]         @ W_g.T  + b_g  + x_g)
    c_t = f_l*c_l + f_r*c_r + i*g
    o   = sigmoid([h_l h_r c_t]     @ W_o.T  + b_o  + x_o)
    h_t = o * tanh(c_t)
returns (x_t, h_t, c_t).

Strategy: data-parallel over 8 NeuronCores (1024 rows each). All work is done
in transposed space (features on SBUF partitions, rows on the free dim) so no
on-device transposes are needed; the host pre-transposes activations and
pre-tiles weights. The x_in projection rides each gate's PSUM accumulation
(x_i in the i-gate, x_g in g, x_o in o; the shared x_f is computed once per
m-tile and DVE-added into both f-gate PSUMs). Matmuls run in bf16 with fp32
PSUM accumulate; elementwise math runs in fp32. Weight streaming uses the
sync HWDGE DMA ring, activation loads the SWDGE ring, and output stores the
scalar HWDGE ring, so the three flows never FIFO-block each other.
"""

import sys

if "/opt/trn_rl_repo" not in sys.path:
    sys.path.insert(0, "/opt/trn_rl_repo")

import numpy as np
import ml_dtypes

N_CORES = 8
N = 8192
D = 1024
P = 128
NL = N // N_CORES          # rows per core
KB = D // P                # 8 k-blocks per 1024-feature tensor
MT = D // P                # 8 output m-tiles per gate

# (name, K-tiles, bias index, act fn, [(rhs part, weight k-tile offset)],
#  xin key). The x_in projection x_t @ W_xin.T is precomputed per m-tile
# into SBUF a few m ahead (it needs no h/c data), then DVE-added into the
# gate PSUM — this gives the PE h/c-independent work to chew on while the
# h/c resident loads stream in.
_GATES_B = [
    ("i",  40, 1, "sig",  [("xtb", 32), ("hbl", 0), ("hbr", 8),
                           ("cbl", 16), ("cbr", 24)], None),
    ("fl", 32, 2, "sig",  [("hbl", 0), ("hbr", 8), ("cbl", 16), ("cbr", 24)], "xf"),
    ("fr", 32, 3, "sig",  [("hbl", 0), ("hbr", 8), ("cbl", 16), ("cbr", 24)], "xf"),
    ("g",  24, 4, "tanh", [("xtb", 16), ("hbl", 0), ("hbr", 8)], None),
]

_compiled = {}


def _build(R):
    """Build + compile the per-core Bass program. R = rows per slab."""
    import concourse.mybir as mybir
    import concourse.tile as tile
    from concourse import bacc

    F32 = mybir.dt.float32
    BF16 = mybir.dt.bfloat16
    SIG = mybir.ActivationFunctionType.Sigmoid
    TANH = mybir.ActivationFunctionType.Tanh

    assert NL % R == 0
    n_slabs = NL // R

    nc = bacc.Bacc("TRN2", target_bir_lowering=False, debug=False)

    def din(name, shape, dt):
        return nc.dram_tensor(name, shape, dt, kind="ExternalInput").ap()

    def dout(name, shape, dt):
        return nc.dram_tensor(name, shape, dt, kind="ExternalOutput").ap()

    # Transposed activations [D, NL]; b-prefix = bf16 (matmul), f = fp32.
    xb_l = din("xb_l", [D, NL], BF16)
    xb_r = din("xb_r", [D, NL], BF16)
    hb_l = din("hb_l", [D, NL], BF16)
    hb_r = din("hb_r", [D, NL], BF16)
    cb_l = din("cb_l", [D, NL], BF16)
    cb_r = din("cb_r", [D, NL], BF16)
    # bf16 residuals: c ~= bf16(c) + bf16(c - bf16(c)) to ~2^-17 relative,
    # reconstructed on the DVE — half the DMA bytes of an fp32 c stream.
    clo_l = din("clo_l", [D, NL], BF16)
    clo_r = din("clo_r", [D, NL], BF16)
    # Weights pre-tiled on host to [MT, P, Kt, P] (partition-major so each
    # per-partition DMA run is Kt*256B contiguous).
    wz = din("wz", [MT, P, 16, P], BF16)
    wi = din("wi", [MT, P, 40, P], BF16)
    wfl = din("wfl", [MT, P, 32, P], BF16)
    wfr = din("wfr", [MT, P, 32, P], BF16)
    wg = din("wg", [MT, P, 24, P], BF16)
    wo = din("wo", [MT, P, 32, P], BF16)
    wxf = din("wxf", [MT, P, 8, P], BF16)
    wmap = {"i": wi, "fl": wfl, "fr": wfr, "g": wg}
    bias = din("bias", [P, 6, MT], F32)

    xT_o = dout("xT_o", [D, NL], F32)
    hT_o = dout("hT_o", [D, NL], F32)
    cT_o = dout("cT_o", [D, NL], F32)

    def r3(ap):
        return ap.rearrange("(k p) n -> p k n", p=P)

    with tile.TileContext(nc) as tc:
        with (
            tc.tile_pool(name="acts", bufs=1) as apool,
            tc.tile_pool(name="w", bufs=4) as wpool,
            tc.tile_pool(name="gates", bufs=8) as gpool,
            tc.tile_pool(name="work", bufs=12) as wkpool,
            tc.tile_pool(name="ps", bufs=8, space="PSUM") as pspool,
            tc.tile_pool(name="cst", bufs=1) as cpool,
        ):
            bias_t = cpool.tile([P, 6, MT], F32, name="bias_t")
            nc.sync.dma_start(bias_t[:], bias[:])

            # PE warmup: the HAM clock gate needs ~3.4us of sustained PE
            # activity to lift the PE clock from 1.2 to 2.4 GHz and drops it
            # back after ~3.4us idle. These dummies (on a memset tile) span
            # the initial DMA window so the first real matmuls run warm.
            warm_t = cpool.tile([P, 512], BF16, name="warm_t")
            nc.vector.memset(warm_t[:], 0.0)
            ps_warm = pspool.tile([P, R], F32, tag="ps", name="ps_warm")
            for _ in range(28):
                nc.tensor.matmul(ps_warm[:, :512], warm_t[:, :P], warm_t[:],
                                 start=True, stop=True)

            def load_xb(s):
                rs_ = slice(s * R, s * R + R)
                # Split the first load so the very first matmul's k-blocks
                # land sooner.
                xl = apool.tile([P, KB, R], BF16, tag="xbl", name="xbl")
                nc.gpsimd.dma_start(xl[:, :2, :], r3(xb_l)[:, :2, rs_])
                nc.gpsimd.dma_start(xl[:, 2:, :], r3(xb_l)[:, 2:, rs_])
                # x_r rides the scalar HWDGE ring, which is idle at slab
                # start, so both phase-A inputs stream concurrently.
                xr = apool.tile([P, KB, R], BF16, tag="xbr", name="xbr")
                nc.scalar.dma_start(xr[:], r3(xb_r)[:, :, rs_])
                return xl, xr

            next_xb = load_xb(0)
            for s in range(n_slabs):
                r0 = s * R
                rs = slice(r0, r0 + R)

                # Activations ride the SWDGE (gpsimd) DMA ring so they never
                # FIFO-block weight streaming on the sync (HWDGE) ring.
                def lda(name, dram, dt):
                    t = apool.tile([P, KB, R], dt, tag=name, name=name)
                    nc.gpsimd.dma_start(t[:], r3(dram)[:, :, rs])
                    return t

                xbl_t, xbr_t = next_xb
                xtb_t = apool.tile([P, KB, R], BF16, tag="xtb", name="xtb")
                ctb_t = apool.tile([P, KB, R], BF16, tag="ctb", name="ctb")
                tct_t = apool.tile([P, KB, R], F32, tag="tct", name="tct")

                # z weights m=5..7 ride the scalar HWDGE queue (own tag so
                # their dma_starts never WAR-stall the scalar engine), which
                # trims the sync queue's pre-phase-B critical path by 1.5 MB
                # and pulls wi[0]'s arrival ahead of the phase-B start.
                wzs = []
                for m in range(5, MT):
                    t = wpool.tile([P, 16, P], BF16, tag="wzs", bufs=3,
                                   name="wzs_t")
                    nc.scalar.dma_start(t[:], wz[m])
                    wzs.append(t)

                # ---- Phase A: z gate + x_t ----
                for m in range(MT):
                    if m >= 5:
                        w_t = wzs[m - 5]
                    else:
                        w_t = wpool.tile([P, 16, P], BF16, tag="w",
                                         name="wz_t")
                        nc.sync.dma_start(w_t[:], wz[m])
                    ps = pspool.tile([P, R], F32, tag="ps", name="ps_z")
                    for kt in range(16):
                        rhs = (xbl_t if kt < KB else xbr_t)[:, kt % KB, :]
                        nc.tensor.matmul(ps[:], w_t[:, kt, :], rhs,
                                         start=(kt == 0), stop=(kt == 15))
                    z_t = wkpool.tile([P, R], F32, tag="wk", name="z_t")
                    nc.scalar.activation(z_t[:], ps[:], SIG, bias=bias_t[:, 0, m, None])
                    d_t = wkpool.tile([P, R], F32, tag="wk", name="d_t")
                    nc.vector.tensor_sub(d_t[:], xbl_t[:, m, :], xbr_t[:, m, :])
                    xrf_m = wkpool.tile([P, R], F32, tag="wk", name="xrf_m")
                    nc.vector.tensor_copy(xrf_m[:], xbr_t[:, m, :])
                    nc.vector.tensor_mul(d_t[:], d_t[:], z_t[:])
                    xt_m = wkpool.tile([P, R], F32, tag="wk", name="xt_m")
                    nc.vector.tensor_add(xt_m[:], d_t[:], xrf_m[:])
                    nc.scalar.dma_start(r3(xT_o)[:, m, rs], xt_m[:])
                    nc.vector.tensor_copy(xtb_t[:, m, :], xt_m[:])

                # Resident loads for phases B/C stream during phase-A compute.
                hbl_t = lda("hbl", hb_l, BF16)
                hbr_t = lda("hbr", hb_r, BF16)
                cbl_t = lda("cbl", cb_l, BF16)
                cbr_t = lda("cbr", cb_r, BF16)
                if s + 1 < n_slabs:
                    next_xb = load_xb(s + 1)
                parts = {"hbl": hbl_t, "hbr": hbr_t, "cbl": cbl_t,
                         "cbr": cbr_t, "xtb": xtb_t, "ctb": ctb_t}
                clol_t = clor_t = None

                # ---- Phase B: i, f_l, f_r, g gates + c_t ----
                for m in range(MT):
                    # x_f pre-activation is shared by f_l and f_r: compute it
                    # once per m-tile and DVE-add it into both gate PSUMs.
                    wxf_t = wpool.tile([P, KB, P], BF16, tag="w", name="wxf_t")
                    nc.sync.dma_start(wxf_t[:], wxf[m])
                    if m == 0:
                        # c residuals are first needed for c_t at the end of
                        # B m=0; loading here keeps them out of the phase-A
                        # window.
                        clol_t = lda("clol", clo_l, BF16)
                        clor_t = lda("clor", clo_r, BF16)
                    ps_xf = pspool.tile([P, R], F32, tag="ps", name="ps_xf")
                    for j in range(KB):
                        nc.tensor.matmul(ps_xf[:], wxf_t[:, j, :],
                                         xtb_t[:, j, :],
                                         start=(j == 0), stop=(j == KB - 1))
                    xfp_m = gpool.tile([P, R], F32, tag="gate", name="xfp_m")
                    nc.scalar.copy(xfp_m[:], ps_xf[:])
                    gt = {}
                    for (gname, Kt, b_idx, fn, rparts, xkey) in _GATES_B:
                        w_t = wpool.tile([P, Kt, P], BF16, tag="w",
                                         name=f"w_{gname}")
                        nc.sync.dma_start(w_t[:], wmap[gname][m])
                        ps = pspool.tile([P, R], F32, tag="ps",
                                         name=f"ps_{gname}")
                        n_done = 0
                        for (pname, koff) in rparts:
                            pt = parts[pname]
                            for j in range(KB):
                                nc.tensor.matmul(
                                    ps[:], w_t[:, koff + j, :], pt[:, j, :],
                                    start=(n_done == 0),
                                    stop=(n_done == Kt - 1))
                                n_done += 1
                        if xkey == "xf":
                            nc.vector.tensor_add(ps[:], ps[:], xfp_m[:])
                        g_t = gpool.tile([P, R], F32, tag="gate",
                                         name=f"g_{gname}")
                        nc.scalar.activation(
                            g_t[:], ps[:], SIG if fn == "sig" else TANH,
                            bias=bias_t[:, b_idx, m, None])
                        gt[gname] = g_t
                    cfl_m = wkpool.tile([P, R], F32, tag="wk", name="cfl_m")
                    nc.vector.tensor_add(cfl_m[:], cbl_t[:, m, :],
                                         clol_t[:, m, :])
                    cfr_m = wkpool.tile([P, R], F32, tag="wk", name="cfr_m")
                    nc.vector.tensor_add(cfr_m[:], cbr_t[:, m, :],
                                         clor_t[:, m, :])
                    ct_m = wkpool.tile([P, R], F32, tag="wk", name="ct_m")
                    nc.vector.tensor_mul(ct_m[:], gt["fl"][:], cfl_m[:])
                    t2 = wkpool.tile([P, R], F32, tag="wk", name="t2")
                    nc.vector.tensor_mul(t2[:], gt["fr"][:], cfr_m[:])
                    nc.vector.tensor_add(ct_m[:], ct_m[:], t2[:])
                    nc.vector.tensor_mul(t2[:], gt["i"][:], gt["g"][:])
                    nc.vector.tensor_add(ct_m[:], ct_m[:], t2[:])
                    nc.scalar.dma_start(r3(cT_o)[:, m, rs], ct_m[:])
                    nc.vector.tensor_copy(ctb_t[:, m, :], ct_m[:])
                    nc.scalar.activation(tct_t[:, m, :], ct_m[:], TANH)

                # ---- Phase C: o gate + h_t ----
                for m in range(MT):
                    w_t = wpool.tile([P, 32, P], BF16, tag="w", name="wo_t")
                    nc.sync.dma_start(w_t[:], wo[m])
                    ps = pspool.tile([P, R], F32, tag="ps", name="ps_o")
                    kt = 0
                    for pname in ["hbl", "hbr", "ctb", "xtb"]:
                        pt = parts[pname]
                        for j in range(KB):
                            nc.tensor.matmul(ps[:], w_t[:, kt, :], pt[:, j, :],
                                             start=(kt == 0), stop=(kt == 31))
                            kt += 1
                    if not (s == n_slabs - 1 and m == MT - 1):
                        o_t = wkpool.tile([P, R], F32, tag="wk", name="o_t")
                        nc.scalar.activation(o_t[:], ps[:], SIG,
                                             bias=bias_t[:, 5, m, None])
                        ht_m = wkpool.tile([P, R], F32, tag="wk", name="ht_m")
                        nc.vector.tensor_mul(ht_m[:], o_t[:], tct_t[:, m, :])
                        nc.scalar.dma_start(r3(hT_o)[:, m, rs], ht_m[:])
                    else:
                        # Tail: the kernel's last serial chain (act -> mul ->
                        # store -> HBM receipt). Two row-halves, the second
                        # store on the now weight-idle sync ring, shortens it.
                        H = R // 2
                        for hi, eng in ((0, nc.scalar), (1, nc.sync)):
                            hs = slice(hi * H, (hi + 1) * H)
                            rh = slice(r0 + hi * H, r0 + (hi + 1) * H)
                            o_h = wkpool.tile([P, H], F32, tag="wkh", bufs=4,
                                              name="o_h")
                            nc.scalar.activation(o_h[:], ps[:, hs], SIG,
                                                 bias=bias_t[:, 5, m, None])
                            ht_h = wkpool.tile([P, H], F32, tag="wkh", bufs=4,
                                               name="ht_h")
                            nc.vector.tensor_mul(ht_h[:], o_h[:],
                                                 tct_t[:, m, hs])
                            eng.dma_start(r3(hT_o)[:, m, rh], ht_h[:])

    nc.compile()
    return nc


def _get_compiled(R=512):
    if R not in _compiled:
        _compiled[R] = _build(R)
    return _compiled[R]


def _prep_weight(w_km):
    """[K, D] (K-major stack of W.T blocks) -> [MT, P, Kt, P] bf16."""
    K = w_km.shape[0]
    kt = K // P
    w = w_km.reshape(kt, P, MT, P)          # [kt, p, m, f]
    w = np.ascontiguousarray(w.transpose(2, 1, 0, 3))  # [m, p, kt, f]
    return w.astype(ml_dtypes.bfloat16)


def _host_prep(inp):
    """Transpose/stack/cast everything the device program wants."""
    f32 = np.float32
    t = {k: np.ascontiguousarray(np.asarray(inp[k], dtype=f32).T)
         for k in ("x_l", "x_r", "h_l", "h_r", "c_l", "c_r")}
    bf = {k: v.astype(ml_dtypes.bfloat16) for k, v in t.items()}
    # c residuals: c ~= bf16(c) + bf16(c - bf16(c)) to ~2^-17 relative.
    clo = {k: (t[k] - bf[k].astype(f32)).astype(ml_dtypes.bfloat16)
           for k in ("c_l", "c_r")}

    W_i = np.asarray(inp["W_i"], f32)
    W_fl = np.asarray(inp["W_fl"], f32)
    W_fr = np.asarray(inp["W_fr"], f32)
    W_xin = np.asarray(inp["W_xin"], f32)
    W_o = np.asarray(inp["W_o"], f32)
    W_z = np.asarray(inp["W_z"], f32)
    W_g = np.asarray(inp["W_g"], f32)

    x_i = W_xin[0 * D:1 * D].T    # [D, D] blocks of W_xin.T
    x_f = W_xin[1 * D:2 * D].T
    x_o = W_xin[2 * D:3 * D].T
    x_g = W_xin[3 * D:4 * D].T

    weights = {
        "wz": _prep_weight(np.ascontiguousarray(W_z.T)),
        "wi": _prep_weight(np.concatenate([W_i.T, x_i], axis=0)),
        "wfl": _prep_weight(np.ascontiguousarray(W_fl.T)),
        "wfr": _prep_weight(np.ascontiguousarray(W_fr.T)),
        "wg": _prep_weight(np.concatenate([W_g.T, x_g], axis=0)),
        "wo": _prep_weight(np.concatenate([W_o.T, x_o], axis=0)),
        "wxf": _prep_weight(x_f),
    }

    b = np.stack([np.asarray(inp[k], f32) for k in
                  ("b_z", "b_i", "b_fl", "b_fr", "b_g", "b_o")])  # [6, D]
    bias = np.ascontiguousarray(b.reshape(6, MT, P).transpose(2, 0, 1))

    in_maps = []
    for c in range(N_CORES):
        cs = slice(c * NL, (c + 1) * NL)
        m = {
            "xb_l": np.ascontiguousarray(bf["x_l"][:, cs]),
            "xb_r": np.ascontiguousarray(bf["x_r"][:, cs]),
            "hb_l": np.ascontiguousarray(bf["h_l"][:, cs]),
            "hb_r": np.ascontiguousarray(bf["h_r"][:, cs]),
            "cb_l": np.ascontiguousarray(bf["c_l"][:, cs]),
            "cb_r": np.ascontiguousarray(bf["c_r"][:, cs]),
            "clo_l": np.ascontiguousarray(clo["c_l"][:, cs]),
            "clo_r": np.ascontiguousarray(clo["c_r"][:, cs]),
            "bias": bias,
        }
        m.update(weights)
        in_maps.append(m)
    return in_maps


def run(inputs, R=512, trace=False, trace_kwargs=None):
    """Run on 8 cores; returns (results, BassKernelResults)."""
    from concourse.bass_utils import run_bass_kernel_spmd

    if trace:
        try:
            from hookfix import install_ntff_hook
            install_ntff_hook()
        except Exception:
            pass
    nc = _get_compiled(R)
    in_maps = _host_prep(inputs)
    res = run_bass_kernel_spmd(nc, in_maps, core_ids=list(range(N_CORES)),
                               trace=trace, **(trace_kwargs or {}))
    xT = np.concatenate([res.results[c]["xT_o"] for c in range(N_CORES)], axis=1)
    hT = np.concatenate([res.results[c]["hT_o"] for c in range(N_CORES)], axis=1)
    cT = np.concatenate([res.results[c]["cT_o"] for c in range(N_CORES)], axis=1)
    x_t = np.ascontiguousarray(xT.T)
    h_t = np.ascontiguousarray(hT.T)
    c_t = np.ascontiguousarray(cT.T)
    return (x_t, h_t, c_t), res


def kernel(**inputs):
    out, _ = run(inputs)
    return out

